# revision 1
# baseline (speedup 1.0000x reference)
"""BotRGCN forward on 8 Trainium2 NeuronCores (Bass/Tile).

Strategy (dst-sharded message passing):
  - nodes sharded 8 ways; each core computes its shard of the fused
    feature projection x = sum_b leaky(BN_b(Linear_b(.))) in transposed
    form (x.T tiles), with BN folded into the linear weights on host.
  - x shards AllGather'd into a full [100000, 64] f32 gather table.
  - edges partitioned by dst shard, sorted by (src_region, dst_block,
    type), padded to 128-slot chunks; per-chunk one-hot matmul
    (lhsT = (iota==dst_local)*w, rhs = gathered msg rows) accumulates
    weighted means per (dst_block, relation) directly in PSUM
    (w = 1/deg precomputed on host -> no division on device).
  - per-block epilogue runs fully transposed (mean_r.T -> relation
    mix -> classifier) so biases ride the ScalarEngine activation.
Messages are gathered with 4-queue SWDGE dma_gather (256B rows, int16
indices relative to 25000-row regions of the table).
"""
import math
import numpy as np

N = 100000
E_H = 64
R = 2
CORES = 8
NS = N // CORES            # 12500 nodes per core
P = 128
NBLK = math.ceil(NS / P)   # 98 dst blocks per core (last partial: 84)
REGN = 25000               # gather region rows (int16-addressable)
NREG = N // REGN           # 4
WIN = 500                  # phase-1 node window
NWIN = NS // WIN           # 25
TCH = WIN // 4             # transpose-back chunk rows
KD = 768                   # des features
KNC = 11                   # num(5) + cat(6)
CALLCH = 64                # max chunks per dma_gather call (8192 idxs)
TBW = 128                  # bf16 gather-table row width (256B rows)
NEG = 0.01
EPS = np.float32(1e-5)

_cache = {}


def _fold(W, b, g, beta, m, v):
    s = (g / np.sqrt(v + EPS)).astype(np.float32)
    return (W * s[:, None]).astype(np.float32), ((b - m) * s + beta).astype(np.float32)


def _preprocess(inputs):
    """Host-side: fold BN, transpose/shard features, build the edge
    schedule + gather/one-hot tables. Pure numpy/index work."""
    src = np.asarray(inputs["edge_index"][0], dtype=np.int64)
    dst = np.asarray(inputs["edge_index"][1], dtype=np.int64)
    et = np.asarray(inputs["edge_type"], dtype=np.int64)
    E = src.shape[0]

    Wd, bd = _fold(np.asarray(inputs["des_W"], dtype=np.float32), np.asarray(inputs["des_b"], dtype=np.float32),
                   np.asarray(inputs["des_g"], dtype=np.float32), np.asarray(inputs["des_beta"], dtype=np.float32),
                   np.asarray(inputs["des_m"], dtype=np.float32), np.asarray(inputs["des_v"], dtype=np.float32))
    Wn, bn = _fold(np.asarray(inputs["num_W"], dtype=np.float32), np.asarray(inputs["num_b"], dtype=np.float32),
                   np.asarray(inputs["num_g"], dtype=np.float32), np.asarray(inputs["num_beta"], dtype=np.float32),
                   np.asarray(inputs["num_m"], dtype=np.float32), np.asarray(inputs["num_v"], dtype=np.float32))
    Wc, bc = _fold(np.asarray(inputs["cat_W"], dtype=np.float32), np.asarray(inputs["cat_b"], dtype=np.float32),
                   np.asarray(inputs["cat_g"], dtype=np.float32), np.asarray(inputs["cat_beta"], dtype=np.float32),
                   np.asarray(inputs["cat_m"], dtype=np.float32), np.asarray(inputs["cat_v"], dtype=np.float32))

    wdT = np.ascontiguousarray(Wd.T)                     # [768, 64]
    wncT = np.zeros((KNC, 128), np.float32)
    wncT[0:5, 0:64] = Wn.T
    wncT[5:11, 64:128] = Wc.T
    bd_t = bd.reshape(E_H, 1)
    bnc_t = np.concatenate([bn, bc]).reshape(128, 1)

    rg = np.asarray(inputs["rgcn_W"], dtype=np.float32)  # [2, 64, 64]
    w0T = np.ascontiguousarray(rg[0].T)
    w1T = np.ascontiguousarray(rg[1].T)
    W1 = np.asarray(inputs["cls_W1"], dtype=np.float32)  # [32, 64]
    W2 = np.asarray(inputs["cls_W2"], dtype=np.float32)  # [2, 32]
    w1cT = np.ascontiguousarray(W1.T)                    # [64, 32]
    w2T = np.ascontiguousarray(W2.T)                     # [32, 2]
    b1_t = np.asarray(inputs["cls_b1"], dtype=np.float32).reshape(32, 1)
    b2rep = np.tile(np.asarray(inputs["cls_b2"], dtype=np.float32)[None, :], (128, 1))

    # --- edge schedule ------------------------------------------------
    shard = dst // NS
    blk = (dst % NS) // P
    dloc = (dst % NS) % P
    reg = src // REGN
    idx16 = (src % REGN).astype(np.int16)
    cnt = np.bincount(dst * R + et, minlength=N * R).astype(np.float32)
    w = (1.0 / np.maximum(cnt, 1.0))[dst * R + et].astype(np.float32)

    # group id: (shard, reg, blk, type) -- region-major within shard
    gid = ((shard * NREG + reg) * NBLK + blk) * R + et
    ngroups = CORES * NREG * NBLK * R
    counts = np.bincount(gid, minlength=ngroups)
    order = np.argsort(gid, kind="stable")
    gstart = np.zeros(ngroups + 1, np.int64)
    np.cumsum(counts, out=gstart[1:])
    rank = np.empty(E, np.int64)
    rank[order] = np.arange(E) - gstart[gid[order]]

    # shared chunk schedule: per (reg, blk, type) max over shards
    cpc = counts.reshape(CORES, NREG, NBLK, R)
    chunks = np.ceil(cpc.max(axis=0) / P).astype(np.int64)   # [NREG, NBLK, R]
    slot_off = np.zeros(NREG * NBLK * R + 1, np.int64)
    np.cumsum(chunks.reshape(-1) * P, out=slot_off[1:])
    S_tot = int(slot_off[-1])
    nchunks_tot = S_tot // P

    # per-core slot tables
    lgid = gid - shard * (NREG * NBLK * R)
    slot = slot_off[lgid] + rank
    idx_slot = np.zeros((CORES, S_tot), np.int16)
    dloc_slot = np.zeros((CORES, S_tot), np.float32)
    w_slot = np.zeros((CORES, S_tot), np.float32)
    idx_slot[shard, slot] = idx16
    dloc_slot[shard, slot] = dloc.astype(np.float32)
    w_slot[shard, slot] = w

    # region pass boundaries in chunks
    reg_bounds = [int(slot_off[g * NBLK * R] // P) for g in range(NREG)] + [nchunks_tot]

    # gather calls: within each region pass, calls of <= CALLCH chunks
    calls = []   # (g, chunk_lo, nch)
    for g in range(NREG):
        lo, hi = reg_bounds[g], reg_bounds[g + 1]
        c = lo
        while c < hi:
            n = min(CALLCH, hi - c)
            calls.append((g, c, n))
            c += n
    ncalls = len(calls)

    # packed gather-idx (wrapped 16 + replicated x8) and dloc/w per call
    idx_pack = np.zeros((CORES, ncalls, 128, CALLCH * 8), np.int16)
    dlw_pack = np.zeros((CORES, ncalls, 128, CALLCH * 2), np.float32)
    ch_idx = idx_slot.reshape(CORES, nchunks_tot, P)
    ch_dloc = dloc_slot.reshape(CORES, nchunks_tot, P)
    ch_w = w_slot.reshape(CORES, nchunks_tot, P)
    for ci, (g, lo, n) in enumerate(calls):
        flat = ch_idx[:, lo:lo + n, :].reshape(CORES, n * P)
        wrap = flat.reshape(CORES, n * P // 16, 16).transpose(0, 2, 1)
        for grp in range(8):
            idx_pack[:, ci, 16 * grp:16 * grp + 16, :n * P // 16] = wrap
        dlw_pack[:, ci, :, 0:2 * n:2] = ch_dloc[:, lo:lo + n, :].transpose(0, 2, 1)
        dlw_pack[:, ci, :, 1:2 * n:2] = ch_w[:, lo:lo + n, :].transpose(0, 2, 1)

    # ordered sub-run list: for each (g, b): [(t, nch, chunk_lo), ...]
    gb_runs = []     # list of (g, b, [(t, nch, lo), ...]) in emission order
    last_g = [-1] * NBLK
    for g in range(NREG):
        for b in range(NBLK):
            tl = []
            for t in range(R):
                nch = int(chunks[g, b, t])
                if nch:
                    lo = int(slot_off[(g * NBLK + b) * R + t] // P)
                    tl.append((t, nch, lo))
            if tl:
                gb_runs.append((g, b, tl))
                last_g[b] = g

    # per-core dense features, transposed + sharded
    des = np.asarray(inputs["des"], dtype=np.float32)
    num = np.asarray(inputs["num"], dtype=np.float32)
    cat = np.asarray(inputs["cat"], dtype=np.float32)
    desT, ncT = [], []
    for k in range(CORES):
        sl = slice(NS * k, NS * (k + 1))
        desT.append(np.ascontiguousarray(des[sl].T))
        ncT.append(np.ascontiguousarray(
            np.concatenate([num[sl], cat[sl]], axis=1).T))

    consts = dict(wdT=wdT, bd=bd_t, wncT=wncT, bnc=bnc_t, w0T=w0T, w1T=w1T,
                  w1cT=w1cT, b1=b1_t, w2T=w2T, b2rep=b2rep)
    sched = dict(calls=calls, gb_runs=gb_runs, last_g=last_g, ncalls=ncalls,
                 nchunks_tot=nchunks_tot)
    percore = dict(desT=desT, ncT=ncT, idx=idx_pack, dlw=dlw_pack)
    return consts, sched, percore


def _build(sched, skip_oh=False, skip_mm=False, skip_ep=False):
    """Emit the Bass/Tile program for the shared schedule."""
    import contextlib
    import concourse.bacc as bacc
    import concourse.mybir as mybir
    import concourse.tile as tile
    from concourse.masks import make_identity

    f32 = mybir.dt.float32
    i16 = mybir.dt.int16
    Lrelu = mybir.ActivationFunctionType.Prelu
    calls, gb_runs, last_g = sched["calls"], sched["gb_runs"], sched["last_g"]
    ncalls = sched["ncalls"]

    nc = bacc.Bacc(num_swdge_queues=4)
    desT_in = nc.dram_tensor("desT", [KD, NS], f32, kind="ExternalInput")
    ncT_in = nc.dram_tensor("ncT", [KNC, NS], f32, kind="ExternalInput")
    wdT_in = nc.dram_tensor("wdT", [KD, E_H], f32, kind="ExternalInput")
    bd_in = nc.dram_tensor("bd", [E_H, 1], f32, kind="ExternalInput")
    wncT_in = nc.dram_tensor("wncT", [KNC, 128], f32, kind="ExternalInput")
    bnc_in = nc.dram_tensor("bnc", [128, 1], f32, kind="ExternalInput")
    w0T_in = nc.dram_tensor("w0T", [E_H, E_H], f32, kind="ExternalInput")
    w1T_in = nc.dram_tensor("w1T", [E_H, E_H], f32, kind="ExternalInput")
    w1cT_in = nc.dram_tensor("w1cT", [E_H, 32], f32, kind="ExternalInput")
    b1_in = nc.dram_tensor("b1", [32, 1], f32, kind="ExternalInput")
    w2T_in = nc.dram_tensor("w2T", [32, 2], f32, kind="ExternalInput")
    b2_in = nc.dram_tensor("b2rep", [128, 2], f32, kind="ExternalInput")
    idx_in = nc.dram_tensor("idx", [ncalls, 128, CALLCH * 8], i16, kind="ExternalInput")
    dlw_in = nc.dram_tensor("dlw", [ncalls, 128, CALLCH * 2], f32, kind="ExternalInput")
    y_out = nc.dram_tensor("y", [NS, 2], f32, kind="ExternalOutput")

    bf16 = mybir.dt.bfloat16
    cc_in = nc.dram_tensor("cc_in", [NS, TBW], bf16)
    cc_out = nc.dram_tensor("cc_out", [N, TBW], bf16, addr_space="Shared")

    with tile.TileContext(nc) as tc:
        with contextlib.ExitStack() as ctx:
            consts = ctx.enter_context(tc.tile_pool(name="consts", bufs=1))
            wdT_t = consts.tile([P, (KD // P) * E_H], f32)
            nc.sync.dma_start(out=wdT_t[:].rearrange("k (c m) -> k c m", m=E_H),
                              in_=wdT_in[:, :].rearrange("(c k) m -> k c m", k=P))
            bd_t = consts.tile([E_H, 1], f32)
            nc.sync.dma_start(out=bd_t[:], in_=bd_in[:, :])
            wncT_t = consts.tile([KNC, 128], f32)
            nc.sync.dma_start(out=wncT_t[:], in_=wncT_in[:, :])
            bnc_t = consts.tile([128, 1], f32)
            nc.sync.dma_start(out=bnc_t[:], in_=bnc_in[:, :])
            w0T_t = consts.tile([E_H, E_H], f32)
            nc.sync.dma_start(out=w0T_t[:], in_=w0T_in[:, :])
            w1T_t = consts.tile([E_H, E_H], f32)
            nc.sync.dma_start(out=w1T_t[:], in_=w1T_in[:, :])
            w1cT_t = consts.tile([E_H, 32], f32)
            nc.sync.dma_start(out=w1cT_t[:], in_=w1cT_in[:, :])
            b1_t = consts.tile([32, 1], f32)
            nc.sync.dma_start(out=b1_t[:], in_=b1_in[:, :])
            w2T_t = consts.tile([32, 2], f32)
            nc.sync.dma_start(out=w2T_t[:], in_=w2T_in[:, :])
            b2_t = consts.tile([128, 2], f32)
            nc.sync.dma_start(out=b2_t[:], in_=b2_in[:, :])
            ident = consts.tile([P, P], f32)
            make_identity(nc, ident[:])
            iota_i = consts.tile([P, P], mybir.dt.int32)
            nc.gpsimd.iota(iota_i[:], pattern=[[1, P]], base=0, channel_multiplier=0)
            iota_f = consts.tile([P, P], mybir.dt.bfloat16)
            nc.vector.tensor_copy(out=iota_f[:], in_=iota_i[:])

            # ---------------- phase 1: projections (x.T windows) -------
            with tc.tile_pool(name="p1sb", bufs=3) as p1sb, \
                 tc.tile_pool(name="p1ps", bufs=2, space="PSUM") as p1ps, \
                 tc.tile_pool(name="p1tp", bufs=4, space="PSUM") as p1tp:
                for wi in range(NWIN):
                    lo = wi * WIN
                    dwin = p1sb.tile([P, (KD // P) * WIN], f32, tag="dwin")
                    nc.sync.dma_start(
                        out=dwin[:].rearrange("k (c n) -> k c n", n=WIN),
                        in_=desT_in[:, lo:lo + WIN].rearrange("(c k) n -> k c n", k=P))
                    ncwin = p1sb.tile([KNC, WIN], f32, tag="ncwin")
                    nc.sync.dma_start(out=ncwin[:], in_=ncT_in[:, lo:lo + WIN])

                    dps = p1ps.tile([E_H, WIN], f32, tag="dps")
                    for c in range(KD // P):
                        nc.tensor.matmul(out=dps[:], lhsT=wdT_t[:, E_H * c:E_H * (c + 1)],
                                         rhs=dwin[:, WIN * c:WIN * (c + 1)],
                                         start=(c == 0), stop=(c == KD // P - 1))
                    ncps = p1ps.tile([128, WIN], f32, tag="ncps")
                    nc.tensor.matmul(out=ncps[:], lhsT=wncT_t[:], rhs=ncwin[:],
                                     start=True, stop=True)

                    xwin = p1sb.tile([E_H, WIN], f32, tag="xwin")
                    nc.scalar.activation(out=xwin[:], in_=dps[:], func=Lrelu,
                                         bias=bd_t[:, :1], alpha=NEG)
                    nc0 = p1sb.tile([E_H, WIN], f32, tag="nc0")
                    nc.scalar.activation(out=nc0[:], in_=ncps[0:E_H, :], func=Lrelu,
                                         bias=bnc_t[0:E_H, :1], alpha=NEG)
                    nc1 = p1sb.tile([E_H, WIN], f32, tag="nc1")
                    nc.scalar.activation(out=nc1[:], in_=ncps[E_H:128, :], func=Lrelu,
                                         bias=bnc_t[E_H:128, :1], alpha=NEG)
                    nc.vector.tensor_add(out=xwin[:], in0=xwin[:], in1=nc0[:])
                    nc.vector.tensor_add(out=xwin[:], in0=xwin[:], in1=nc1[:])

                    xrows = p1sb.tile([TCH, 4 * TBW], mybir.dt.bfloat16, tag="xrows")
                    nc.vector.memset(xrows[:], 0.0)
                    for c in range(4):
                        tp = p1tp.tile([TCH, E_H], f32, tag="tp")
                        nc.tensor.transpose(out=tp[:], in_=xwin[:, TCH * c:TCH * (c + 1)],
                                            identity=ident[0:E_H, 0:E_H])
                        nc.vector.tensor_copy(out=xrows[:, TBW * c:TBW * c + E_H], in_=tp[:])
                    nc.sync.dma_start(
                        out=cc_in[lo:lo + WIN, :].rearrange("(c p) f -> p c f", p=TCH),
                        in_=xrows[:].rearrange("p (c f) -> p c f", f=TBW))

            # ---------------- exchange ---------------------------------
            nc.gpsimd.collective_compute(
                "AllGather", mybir.AluOpType.bypass,
                ins=[cc_in[:, :]], outs=[cc_out[:, :]],
                replica_groups=[list(range(CORES))])

            # ---------------- phase 2: aggregation + epilogue ----------
            with tc.tile_pool(name="accp", bufs=1) as accp, \
                 tc.tile_pool(name="gsb", bufs=4) as gsb, \
                 tc.tile_pool(name="ohp", bufs=6) as ohp, \
                 tc.tile_pool(name="aggps", bufs=3, space="PSUM") as aggps, \
                 tc.tile_pool(name="epips", bufs=4, space="PSUM") as epips, \
                 tc.tile_pool(name="episb", bufs=4) as episb:
                acc = []
                for b in range(NBLK):
                    a = accp.tile([P, 2 * E_H], f32, tag=f"acc{b}")
                    nc.vector.memset(a[:], 0.0)
                    acc.append(a)
                stage = accp.tile([P, 2 * NBLK], f32, tag="stage")

                chunk_pos = {}
                for ci, (g, lo, nch) in enumerate(calls):
                    for c in range(nch):
                        chunk_pos[lo + c] = (ci, c)
                call_tiles = {}

                def emit_call(ci):
                    g, lo, nch = calls[ci]
                    it = gsb.tile([128, CALLCH * 8], i16, tag="it")
                    nc.sync.dma_start(out=it[:], in_=idx_in[ci, :, :])
                    dlw = gsb.tile([128, CALLCH * 2], f32, tag="dlw")
                    nc.sync.dma_start(out=dlw[:], in_=dlw_in[ci, :, :])
                    dst = gsb.tile([P, CALLCH * TBW], mybir.dt.bfloat16, tag="dst")
                    nc.gpsimd.dma_gather(
                        out_ap=dst[:, :nch * TBW].rearrange("p (s f) -> p s f", f=TBW),
                        in_ap=cc_out[REGN * g:REGN * (g + 1), :],
                        idxs_ap=it[:, :],
                        num_idxs=nch * P, num_idxs_reg=nch * P,
                        elem_size=TBW, single_packet=False, queue_num=ci % 4)
                    call_tiles[ci] = (it, dlw, dst)

                def epilogue(b):
                    prows = P if b < NBLK - 1 else NS - P * (NBLK - 1)
                    mrT = epips.tile([E_H, 2 * P], f32, tag="ep")
                    for t in range(R):
                        nc.tensor.transpose(out=mrT[:, P * t:P * (t + 1)],
                                            in_=acc[b][:, E_H * t:E_H * (t + 1)],
                                            identity=ident[:])
                    mrT_s = episb.tile([E_H, 2 * P], f32, tag="mrT_s")
                    nc.vector.tensor_copy(out=mrT_s[:], in_=mrT[:])
                    a2 = epips.tile([E_H, P], f32, tag="ep")
                    nc.tensor.matmul(out=a2[:], lhsT=w0T_t[:], rhs=mrT_s[:, 0:P],
                                     start=True, stop=False)
                    nc.tensor.matmul(out=a2[:], lhsT=w1T_t[:], rhs=mrT_s[:, P:2 * P],
                                     start=False, stop=True)
                    x2 = episb.tile([E_H, P], f32, tag="x2")
                    nc.scalar.activation(out=x2[:], in_=a2[:], func=Lrelu,
                                         scale=0.5, alpha=NEG)
                    hps = epips.tile([32, P], f32, tag="ep")
                    nc.tensor.matmul(out=hps[:], lhsT=w1cT_t[:], rhs=x2[:],
                                     start=True, stop=True)
                    hs = episb.tile([32, P], f32, tag="hs")
                    nc.scalar.activation(out=hs[:], in_=hps[:], func=Lrelu,
                                         bias=b1_t[:, :1], alpha=NEG)
                    ops = epips.tile([P, 2], f32, tag="ep")
                    nc.tensor.matmul(out=ops[:], lhsT=hs[:], rhs=w2T_t[:],
                                     start=True, stop=True)
                    nc.vector.tensor_add(out=stage[:prows, 2 * b:2 * b + 2],
                                         in0=ops[:prows, :], in1=b2_t[:prows, :])

                emitted = set()
                for ri, (g, b, tl) in enumerate(gb_runs):
                    ps = aggps.tile([P, 2 * E_H], f32, tag="agg")
                    touched = [False, False]
                    for (t, nch, lo) in tl:
                        touched[t] = True
                        for c in range(nch):
                            ci, off = chunk_pos[lo + c]
                            if ci not in emitted:
                                emit_call(ci)
                                emitted.add(ci)
                            it, dlw, dst = call_tiles[ci]
                            if skip_oh:
                                oh = iota_f
                            else:
                                oh = ohp.tile([P, P], mybir.dt.bfloat16, tag="oh")
                                nc.vector.tensor_scalar(
                                    out=oh[:], in0=iota_f[:],
                                    scalar1=dlw[:, 2 * off:2 * off + 1],
                                    scalar2=dlw[:, 2 * off + 1:2 * off + 2],
                                    op0=mybir.AluOpType.is_equal,
                                    op1=mybir.AluOpType.mult)
                            if not skip_mm:
                                nc.tensor.matmul(
                                    out=ps[:, E_H * t:E_H * (t + 1)], lhsT=oh[:],
                                    rhs=dst[:, TBW * off:TBW * off + E_H],
                                    start=(c == 0), stop=(c == nch - 1))
                    if not skip_mm:
                        if touched[0] and touched[1]:
                            nc.vector.tensor_add(out=acc[b][:], in0=acc[b][:], in1=ps[:])
                        elif touched[0]:
                            nc.vector.tensor_add(out=acc[b][:, 0:E_H],
                                                 in0=acc[b][:, 0:E_H], in1=ps[:, 0:E_H])
                        else:
                            nc.vector.tensor_add(out=acc[b][:, E_H:2 * E_H],
                                                 in0=acc[b][:, E_H:2 * E_H],
                                                 in1=ps[:, E_H:2 * E_H])
                    if g == last_g[b] and not skip_ep:
                        epilogue(b)
                for b in range(NBLK):
                    if last_g[b] < 0 and not skip_ep:
                        epilogue(b)

                # final output DMA (rows = dst): y[128b + p, j] = stage[p, 2b+j]
                nc.sync.dma_start(
                    out=y_out[0:P * (NBLK - 1), :].rearrange("(b p) j -> p b j", p=P),
                    in_=stage[:, 0:2 * (NBLK - 1)].rearrange("p (b j) -> p b j", j=2))
                tail = NS - P * (NBLK - 1)
                nc.sync.dma_start(
                    out=y_out[P * (NBLK - 1):NS, :],
                    in_=stage[0:tail, 2 * (NBLK - 1):2 * NBLK])

    nc.compile()
    return nc


def _run(nc, consts, sched, percore):
    from concourse.bass_utils import run_bass_kernel_spmd
    in_maps = []
    for k in range(CORES):
        m = dict(desT=percore["desT"][k], ncT=percore["ncT"][k],
                 idx=percore["idx"][k], dlw=percore["dlw"][k])
        m.update({kk: vv for kk, vv in consts.items()})
        in_maps.append(m)
    res = run_bass_kernel_spmd(nc, in_maps, list(range(CORES)))
    return np.concatenate([res.results[k]["y"] for k in range(CORES)], axis=0)


def kernel(**inputs):
    consts, sched, percore = _preprocess(inputs)
    key = (sched["ncalls"], sched["nchunks_tot"],
           tuple((g, b, tuple(tl)) for g, b, tl in sched["gb_runs"]))
    if key not in _cache:
        _cache.clear()
        _cache[key] = _build(sched)
    return _run(_cache[key], consts, sched, percore)

